# revision 24
# baseline (speedup 1.0000x reference)
"""Trainium2 Bass kernel for nn_Head_72507637891886.

Computes r = exp(-(|k|_F^2+|q|_F^2)/2) * mean(cosh((k+q) @ w), -1) where
k = x@wk+bk, q = x@wq+bq, w = sqrt(32) * w_raw.T / |w_raw|_F.

Strategy: data-parallel over batch (2 batches = 8192 tokens per core, 8 cores).
Host pre-transposes each shard to [E=1024, 8192] and quantizes to fp8-e4m3 so
the contraction dim lands on SBUF partitions at 1 byte/element (the kernel is
DMA-bound; fp8 quarters input traffic vs f32 and enables DoubleRow matmuls).

The stationary operand fuses everything the PE needs per token block:
  cols 0:64  = [wk|wq]          -> kq   (only used for the Frobenius norms)
  cols 64:72 = [wz@w | -wz@w]   -> y,-y (wz = wk+wq; exp args for cosh)
since (x@wkq + bkq) @ ws8 = x@(wkq@ws8) + bkq@ws8 — the bias term goes into
the Exp activation's per-partition bias instead of a second matmul.

Per 512-token block on device (engines balanced so the x DMA is the only
saturated resource):
  - PE: bias matmul (ones-row x bias-row, so PSUM starts at +bias) then
    4 DoubleRow fp8 matmuls (0.5 cycles/row) -> [80, 512] PSUM (72 live)
  - DVE: tensor_tensor_reduce square+sum rows 0:64 -> sum-of-squares column
  - ACT: Exp rows 64:72 -> [e^y; e^-y] SBUF
  - Pool: axis-C reduce of the 8 exps -> result row (x0.125 on host)
Host gathers, all-reduces the sum-of-squares scalar, applies the exp factor.
"""

import numpy as np

B, T, E, D = 16, 4096, 1024, 32
OMEGA = 4
NCORES = 8
TOK = B * T // NCORES  # 8192 tokens per core
BLK = 512              # tokens per block (PSUM bank = 512 f32)
NB = TOK // BLK        # 16 blocks
KC = E // 128          # 8 contraction chunks
NST = 2 * D + 2 * OMEGA  # 72 live stationary columns
NSTP = 80  # padded to %16==0 stride: DoubleRow Ldweights requires pair-dim step%16==0

_CACHE = {}
LAST_RESULTS = None  # BassKernelResults from the most recent run (for test.py)
LAST_PROFILE = None
LAST_OUTS = None
TRACE = False


def _build_bass():
    import concourse.bass as bass
    import concourse.mybir as mybir
    import concourse.tile as tile
    from concourse import bacc

    f32 = mybir.dt.float32
    bf16 = mybir.dt.bfloat16
    f32r = mybir.dt.float32r
    f8 = mybir.dt.float8e4
    AF = mybir.ActivationFunctionType
    DR = mybir.MatmulPerfMode.DoubleRow

    nc = bacc.Bacc()
    xt = nc.declare_dram_parameter("xt", [E, TOK], f8, isOutput=False)
    wst = nc.declare_dram_parameter("wst", [128, KC, NSTP], f8, isOutput=False)
    cst8 = nc.declare_dram_parameter("cst8", [1, NSTP + BLK], f8, isOutput=False)
    rout = nc.declare_dram_parameter("rout", [1, TOK], f32, isOutput=True)
    ssout = nc.declare_dram_parameter("ssout", [2 * D, NB + 1], f32, isOutput=True)

    with tile.TileContext(nc) as tc:
        with (
            tc.tile_pool(name="const", bufs=1) as const,
            tc.tile_pool(name="xp", bufs=6) as xp,
            tc.tile_pool(name="work", bufs=4) as work,
            tc.tile_pool(name="acc", bufs=1) as acc,
            tc.tile_pool(name="kqps", bufs=6, space="PSUM") as kqps,
            tc.tile_pool(name="wps", bufs=1, space="PSUM") as wps,
        ):
            # PE p-state warmup: the ramp clock needs ~3us of near-continuous
            # PE activity before matmuls hit full clock (it resets on long
            # idles), so run a back-to-back dummy stream that ends right as
            # the first x block lands; it overlaps the x0 DMA entirely
            warm_sb = const.tile([1, 16], f8)
            nc.gpsimd.memset(warm_sb, 0)
            warm_z = const.tile([1, BLK], f8)
            nc.gpsimd.memset(warm_z, 0)
            warm_ps = wps.tile([16, BLK], f32)
            for _ in range(7):
                nc.tensor.matmul(warm_ps, warm_sb, warm_z, start=True, stop=True)

            x_tiles = {}
            def fetch_x(ib):
                x_t = xp.tile([128, KC, BLK], f8, tag="xt")
                x_tiles[ib] = x_t
                nc.sync.dma_start(
                    out=x_tiles[ib],
                    in_=xt[:, bass.ts(ib, BLK)].rearrange("(c p) t -> p c t", p=128),
                )
            # x0 leads: its 1456ns transfer is the critical-path head; the
            # two const transfers (228ns + 7ns) slot in behind it
            fetch_x(0)
            wst_sb = const.tile([128, KC, NSTP], f8)
            nc.sync.dma_start(out=wst_sb, in_=wst[:])
            cst8_sb = const.tile([1, NSTP + BLK], f8)
            nc.sync.dma_start(out=cst8_sb, in_=cst8[:])
            biasp_sb = cst8_sb[:, :NSTP]
            ones_sb = cst8_sb[:, NSTP:]
            fetch_x(1)

            ss_cols = acc.tile([2 * D, NB + 1], f32)
            r_sb = acc.tile([1, TOK], f32)

            for ib in range(NB):
                tok = bass.ts(ib, BLK)
                if ib + 2 < NB:
                    fetch_x(ib + 2)
                x_tile = x_tiles.pop(ib)

                # bias first: doesn't depend on the x DMA, so PE pre-fills
                # PSUM with [bkq; by] while the transfer is in flight
                kq_ps = kqps.tile([NSTP, BLK], f32)
                nc.tensor.matmul(
                    kq_ps, biasp_sb, ones_sb, start=True, stop=False
                )
                for c in range(KC // 2):
                    nc.tensor.matmul(
                        kq_ps,
                        wst_sb[:, 2 * c : 2 * c + 2, :],
                        x_tile[:, 2 * c : 2 * c + 2, :],
                        start=False,
                        stop=(c == KC // 2 - 1),
                        perf_mode=DR,
                    )

                # Exp first: the result path (exp -> mean -> copy -> rout)
                # is the end-of-program critical chain; the squares only feed
                # the tiny ssout transfer
                e_sb = work.tile([2 * OMEGA, BLK], f32r, tag="esb")
                nc.scalar.activation(
                    e_sb, kq_ps[2 * D : NST, :], AF.Exp, bias=0.0
                )

                # (k+bk)^2 and (q+bq)^2: ACT squares (bias is already in
                # PSUM from the bias matmul), DVE sums along tokens; ACT
                # carries Exp 612ns + Square 612ns per 1456ns DMA cadence.
                # The final block's square runs as two half-token ops so the
                # last ssout column is ready sooner (its DMA chain is one of
                # the two end-of-program critical paths).
                if ib < NB - 1:
                    sq = work.tile([2 * D, BLK], bf16, tag="sqdump")
                    nc.scalar.activation(
                        sq, kq_ps[: 2 * D, :], AF.Square, bias=0.0
                    )
                    nc.vector.tensor_reduce(
                        ss_cols[:, ib : ib + 1], sq,
                        axis=mybir.AxisListType.X, op=mybir.AluOpType.add,
                    )
                else:
                    for h in range(2):
                        half = slice(h * (BLK // 2), (h + 1) * (BLK // 2))
                        sq = work.tile([2 * D, BLK // 2], bf16, tag=f"sqh{h}")
                        nc.scalar.activation(
                            sq, kq_ps[: 2 * D, half], AF.Square, bias=0.0
                        )
                        nc.vector.tensor_reduce(
                            ss_cols[:, ib + h : ib + h + 1], sq,
                            axis=mybir.AxisListType.X, op=mybir.AluOpType.add,
                        )

                # sum of the 8 exps on the otherwise-idle Pool engine,
                # straight into the result row (the 1/8 mean factor rides on
                # the host-side exp(-ss/2) scalar, which multiplies r anyway)
                nc.gpsimd.tensor_reduce(
                    r_sb[:, tok], e_sb,
                    axis=mybir.AxisListType.C, op=mybir.AluOpType.add,
                )

            # final transfers on separate queues so their issue+DGE chains
            # overlap at the end of the program
            nc.sync.dma_start(out=rout[:], in_=r_sb)
            nc.sync.dma_start(out=ssout[:], in_=ss_cols)
    nc.compile()
    return nc


def _get_nc():
    if "nc" not in _CACHE:
        _CACHE["nc"] = _build_bass()
    return _CACHE["nc"]


def kernel(x, wq, bq, wk, bk, wv, bv, w_raw):
    global LAST_RESULTS, LAST_OUTS
    import ml_dtypes
    from concourse.bass_utils import run_bass_kernel_spmd

    f8 = ml_dtypes.float8_e4m3
    x = np.asarray(x, dtype=np.float32)
    wq = np.asarray(wq, dtype=np.float32)
    bq = np.asarray(bq, dtype=np.float32)
    wk = np.asarray(wk, dtype=np.float32)
    bk = np.asarray(bk, dtype=np.float32)
    w_raw = np.asarray(w_raw, dtype=np.float32)

    # replicated small operands
    wt = w_raw.T.astype(np.float32)  # [D, OMEGA]
    norm = np.sqrt(np.sum(wt ** 2, dtype=np.float32))
    w = (np.float32(np.sqrt(np.float32(D))) * (wt / norm)).astype(np.float32)

    wkq = np.concatenate([wk, wq], axis=1)          # [E, 64]
    wy4 = (wk + wq) @ w                             # [E, 4]
    pad = np.zeros((E, NSTP - NST), dtype=np.float32)
    wst_full = np.concatenate([wkq, wy4, -wy4, pad], axis=1)  # [E, 80]
    wst = np.ascontiguousarray(
        wst_full.reshape(KC, 128, NSTP).transpose(1, 0, 2)
    ).astype(f8)  # [128, KC, 80]

    bkq = np.concatenate([bk, bq])                  # [64]
    by4 = (bk + bq) @ w                             # [4]
    cst8 = np.zeros((1, NSTP + BLK), dtype=np.float32)
    cst8[0, :NST] = np.concatenate([bkq, by4, -by4])
    cst8[0, NSTP:] = 1.0
    cst8 = cst8.astype(f8)

    in_maps = []
    bpc = B // NCORES
    for c in range(NCORES):
        xt = np.ascontiguousarray(
            x[c * bpc : (c + 1) * bpc].reshape(TOK, E).T
        ).astype(f8)  # [E, TOK]
        in_maps.append({"xt": xt, "wst": wst, "cst8": cst8})

    nc = _get_nc()
    res = run_bass_kernel_spmd(
        nc, in_maps, core_ids=list(range(NCORES)), trace=False
    )
    LAST_RESULTS = res
    results = res.results
    LAST_OUTS = results

    r_parts = []
    ss = 0.0
    for out in results:
        r_parts.append(out["rout"].reshape(TOK))
        ss += float(out["ssout"].sum(dtype=np.float64))

    # rout holds sum(exp(.)) over the 8 features; 0.125 makes it the mean
    with np.errstate(under="ignore"):
        a = np.float32(0.125 * np.exp(np.float64(-ss / 2.0)))
    r = (a * np.concatenate(r_parts)).reshape(B, T).astype(np.float32)
    return r
